# revision 1
# baseline (speedup 1.0000x reference)
"""Trainium2 Bass kernel for DBHDSNet multi-task detection loss.

Strategy (pure data parallel, B=16 over 8 cores, 2 samples/core):
 - Host (gt-only prep): target assignment replicated with jnp (bit-exact with
   the reference's build_targets on this backend), producing per-core padded
   positive-cell tables + gather row indices; zero-FLOP relayout of reg/cls
   predictions to (cell-major, channel) so positive rows are contiguous.
 - Device: all prediction-dependent math.
   * seg loss (the memory-bound bulk, 13.1MB/core): streaming chunks,
     softplus via Exp/Ln(x+1), sigma via Exp(-L); fused DVE STT reductions
     with accum_out; per-sample sum(t) via PE ones-matmul into PSUM.
   * obj dense softplus sums; positive-cell obj/reg/cls gathered with
     indirect row DMAs; CIoU + focal batched across the 3 scales.
   * hazard CE + expected-cost on [2,4].
 - Each core returns a [128,48] partial-sum tile; host does the final scalar
   reduction (the "all-reduce" of scalar losses).
"""
import sys
sys.path.insert(0, "/opt/trn_rl_repo")
import numpy as np

NUM_CLASSES = 38
GAMMA, ALPHA = 1.5, 0.25
EPS = 1e-7
L_BOX, L_OBJ, L_CLS, L_SEG, L_HAZ, L_HIER = 5.0, 1.0, 1.0, 2.0, 1.0, 0.5
SCALE_RANGES = [(0.0, 0.15), (0.1, 0.35), (0.25, 1.0)]
PENALTY = np.array([[0., 1., 2., 4.], [2., 0., 1., 2.],
                    [4., 2., 0., 1.], [8., 4., 2., 0.]], dtype=np.float32)
B = 16
NCORES = 8
BL = B // NCORES
HWS = [6400, 1600, 400]
DIMS = [(80, 80), (40, 40), (20, 20)]
K = 128
SEG_F = 6400
CHUNK = 1600
NCH = SEG_F // CHUNK
SEG_ELEMS = 32 * 160 * 160
NCOL = 48
CN = NUM_CLASSES

_CACHE = {}


def _build_program():
    import os
    SEC = "all"
    ABL = {"act", "dve", "pe"}
    import concourse.bacc as bacc
    import concourse.bass as bass
    import concourse.mybir as mybir
    import concourse.tile as tile
    dt = mybir.dt
    AF = mybir.ActivationFunctionType
    OP = mybir.AluOpType

    # Route every Exp/Ln to the one table that holds both, so the act-table
    # pass emits a single load instead of one per Exp<->Ln transition.
    from concourse.hw_specs import get_activation_tables as _gat

    def _patched_tables(arch):
        tabs = _gat(arch)
        for name, s in tabs.items():
            if name != "natural_log_exp_and_others":
                s.discard(AF.Exp)
                s.discard(AF.Ln)
        return tabs
    bacc.get_activation_tables = _patched_tables

    nc = bacc.Bacc("TRN2", target_bir_lowering=False, debug=False,
                   num_devices=NCORES)

    def din(name, shape, dty=dt.float32):
        return nc.dram_tensor(name, shape, dty, kind="ExternalInput").ap()

    seg_x = din("seg_x", [BL, 128, SEG_F])
    seg_t = din("seg_t", [BL, 128, SEG_F])
    objd = [din("objd3", [128, 100]), din("objd4", [128, 25]),
            din("objd5", [100, 8])]
    clst = [din(f"clst{i}", [BL * HWS[i], CN]) for i in range(3)]
    regt = [din(f"regt{i}", [BL * HWS[i], 4]) for i in range(3)]
    objf = [din(f"objf{i}", [BL * HWS[i], 1]) for i in range(3)]
    idxs = [din(f"idx{i}", [K, 1], dt.int32) for i in range(3)]
    tboxall = din("tboxall", [K, 12])      # col = ch*3 + scale
    tclsall = din("tclsall", [K, 3 * CN])  # scale-contiguous
    atmall = din("atmall", [K, 3 * CN])
    bmaskall = din("bmaskall", [K, 3])
    haz_x = din("haz_x", [BL, 4])
    haz_oh = din("haz_oh", [BL, 4])
    haz_pr = din("haz_pr", [BL, 4])

    out = nc.dram_tensor("partials", [128, NCOL], dt.float32,
                         kind="ExternalOutput").ap()

    with tile.TileContext(nc) as tc:
        with tc.tile_pool(name="acc", bufs=1) as accp, \
             tc.tile_pool(name="xin", bufs=4) as xp, \
             tc.tile_pool(name="tin", bufs=4) as tp, \
             tc.tile_pool(name="work", bufs=3) as wp, \
             tc.tile_pool(name="junk", bufs=2) as jp, \
             tc.tile_pool(name="small", bufs=1) as sp, \
             tc.tile_pool(name="psum", bufs=2, space="PSUM") as pp:

            acc = accp.tile([128, NCOL], dt.float32)
            nc.vector.memset(acc[:], 0.0)
            ones = sp.tile([128, 1], dt.float32)
            nc.vector.memset(ones[:], 1.0)

            # per-quantity accumulators (avoid cross-engine false deps)
            a_sp = sp.tile([128, 8], dt.float32)    # ACT: sum softplus
            nc.vector.memset(a_sp[:], 0.0)
            a_r = sp.tile([128, 8], dt.float32)     # ACT: sum (1-sigma)
            nc.vector.memset(a_r[:], 0.0)
            a_xt = sp.tile([128, 8], dt.float32)    # DVE: sum x*t
            nc.vector.memset(a_xt[:], 0.0)
            a_rt = sp.tile([128, 8], dt.float32)    # DVE: sum (1-sigma)*t
            nc.vector.memset(a_rt[:], 0.0)
            a_ts = sp.tile([1, 2], dt.float32)      # DVE: sum t per sample
            a_ob = sp.tile([128, 3], dt.float32)    # ACT: obj softplus sums
            nc.vector.memset(a_ob[:], 0.0)
            a_ms = sp.tile([128, 8], dt.float32)    # DVE: objpos 0-2, ciou 3-5
            a_fc = sp.tile([128, 3], dt.float32)    # DVE: focal sums
            a_hz = sp.tile([BL, 2], dt.float32)     # DVE: hazard ce, hier

            # ---------------- seg main loop ----------------
            pst = [pp.tile([1, 512], dt.float32, name=f"pst{s}",
                           tag=f"pst{s}") for s in range(BL)]
            pending = None  # (r_tile, t_tile, col) for pipelined j2
            for s in range(BL):
                for c in range(NCH):
                    xt = xp.tile([128, CHUNK], dt.float32, tag="x")
                    nc.sync.dma_start(xt[:], seg_x[s, :, c*CHUNK:(c+1)*CHUNK])
                    tt = tp.tile([128, CHUNK], dt.float32, tag="t")
                    nc.sync.dma_start(tt[:], seg_t[s, :, c*CHUNK:(c+1)*CHUNK])
                    col = s * NCH + c
                    e = wp.tile([128, CHUNK], dt.float32, tag="e")
                    if "act" not in ABL:
                        continue
                    nc.scalar.activation(e[:], xt[:], AF.Exp)
                    L = wp.tile([128, CHUNK], dt.float32, tag="L")
                    nc.scalar.activation(L[:], e[:], AF.Ln, bias=1.0,
                                         accum_out=a_sp[:, col:col+1])
                    r = wp.tile([128, CHUNK], dt.float32, tag="r")
                    nc.scalar.activation(r[:], L[:], AF.Exp, scale=-1.0,
                                         accum_out=a_r[:, col:col+1])
                    j1 = jp.tile([128, CHUNK], dt.float32, tag="j1")
                    if "dve" in ABL:
                        nc.vector.scalar_tensor_tensor(
                            out=j1[:], in0=xt[:], scalar=1.0, in1=tt[:],
                            op0=OP.mult, op1=OP.mult,
                            accum_out=a_xt[:, col:col+1])
                    # j2 for the PREVIOUS chunk: its r is ready, so the DVE
                    # stream never stalls behind this chunk's 3-deep ACT chain
                    if pending is not None and "dve" in ABL:
                        pr, ptt, pcol = pending
                        j2 = jp.tile([128, CHUNK], dt.float32, tag="j2")
                        nc.vector.scalar_tensor_tensor(
                            out=j2[:], in0=pr[:], scalar=1.0, in1=ptt[:],
                            op0=OP.mult, op1=OP.mult,
                            accum_out=a_rt[:, pcol:pcol+1])
                    pending = (r, tt, col)
                    if "pe" in ABL:
                        nfull = CHUNK // 512
                        for j in range(nfull):
                            nc.tensor.matmul(pst[s][:, :512], lhsT=ones[:],
                                             rhs=tt[:, j*512:(j+1)*512],
                                             start=(c == 0 and j == 0),
                                             stop=False)
                        rem = CHUNK - nfull * 512
                        nc.tensor.matmul(pst[s][:, :rem], lhsT=ones[:],
                                         rhs=tt[:, nfull*512:CHUNK],
                                         start=False, stop=(c == NCH - 1))
            if "dve" in ABL:
                pr, ptt, pcol = pending
                j2 = jp.tile([128, CHUNK], dt.float32, tag="j2")
                nc.vector.scalar_tensor_tensor(
                    out=j2[:], in0=pr[:], scalar=1.0, in1=ptt[:],
                    op0=OP.mult, op1=OP.mult,
                    accum_out=a_rt[:, pcol:pcol+1])
            if "pe" in ABL:
                for s in range(BL):
                    nc.vector.tensor_reduce(a_ts[0:1, s:s+1], pst[s][:],
                                            axis=mybir.AxisListType.X,
                                            op=OP.add)

            # ---------------- obj dense softplus ----------------
            for i in range(3 if SEC in ("all", "obj", "sparse") else 0):
                p, f = objd[i].shape
                ot = sp.tile([p, f], dt.float32, tag=f"objd{i}")
                nc.sync.dma_start(ot[:], objd[i][:])
                oe = sp.tile([p, f], dt.float32, tag=f"obje{i}")
                nc.scalar.activation(oe[:], ot[:], AF.Exp)
                ol = sp.tile([p, f], dt.float32, tag=f"objl{i}")
                nc.scalar.activation(ol[:], oe[:], AF.Ln, bias=1.0,
                                     accum_out=a_ob[0:p, i:i+1])

            # ---------------- gathers + batched sparse ----------------
            if SEC in ("all", "sparse"):
                tba = sp.tile([K, 12], dt.float32)
                nc.sync.dma_start(tba[:], tboxall[:])
                tca = sp.tile([K, 3 * CN], dt.float32)
                nc.sync.dma_start(tca[:], tclsall[:])
                ama = sp.tile([K, 3 * CN], dt.float32)
                nc.sync.dma_start(ama[:], atmall[:])
                bma = sp.tile([K, 3], dt.float32)
                nc.sync.dma_start(bma[:], bmaskall[:])

                gca = sp.tile([K, 3 * CN], dt.float32)
                goa = sp.tile([K, 3], dt.float32)
                d4 = sp.tile([K, 12], dt.float32)
                for i in range(3):
                    it = sp.tile([K, 1], dt.int32, tag=f"idx{i}")
                    nc.sync.dma_start(it[:], idxs[i][:])
                    gr = sp.tile([K, 4], dt.float32, tag=f"gr{i}")
                    nc.gpsimd.indirect_dma_start(
                        out=gr[:], out_offset=None, in_=regt[i][:],
                        in_offset=bass.IndirectOffsetOnAxis(ap=it[:], axis=0))
                    nc.gpsimd.indirect_dma_start(
                        out=gca[:, i*CN:(i+1)*CN], out_offset=None,
                        in_=clst[i][:],
                        in_offset=bass.IndirectOffsetOnAxis(ap=it[:], axis=0))
                    nc.gpsimd.indirect_dma_start(
                        out=goa[:, i:i+1], out_offset=None, in_=objf[i][:],
                        in_offset=bass.IndirectOffsetOnAxis(ap=it[:], axis=0))
                    # sigmoid decode step 1: exp(-x), scale-interleaved cols
                    nc.scalar.activation(d4[:, i:12:3], gr[:], AF.Exp,
                                         scale=-1.0)

                # obj positive sums (all scales at once)
                nc.vector.tensor_tensor(out=a_ms[:, 0:3], in0=goa[:],
                                        in1=bma[:], op=OP.mult)

                # ---- decode finish: dec = 1/(1+exp(-x)), [K,12] ----
                nc.vector.tensor_scalar_add(d4[:], d4[:], 1.0)
                nc.vector.reciprocal(d4[:], d4[:])

                # ---- CIoU batched over 3 scales: ops on [K,3] ----
                w = sp.tile([K, 72], dt.float32)

                def col(n):
                    return w[:, 3*n:3*n+3]
                px, py, pw, ph = (d4[:, 3*m:3*m+3] for m in range(4))
                tx, ty, tw, th = (tba[:, 3*m:3*m+3] for m in range(4))
                phw, phh, thw, thh = col(0), col(1), col(2), col(3)
                nc.vector.tensor_scalar_mul(phw, pw, 0.5)
                nc.vector.tensor_scalar_mul(phh, ph, 0.5)
                nc.vector.tensor_scalar_mul(thw, tw, 0.5)
                nc.vector.tensor_scalar_mul(thh, th, 0.5)
                px1, px2, py1, py2 = col(4), col(5), col(6), col(7)
                nc.vector.tensor_sub(px1, px, phw)
                nc.vector.tensor_add(px2, px, phw)
                nc.vector.tensor_sub(py1, py, phh)
                nc.vector.tensor_add(py2, py, phh)
                tx1, tx2, ty1, ty2 = col(8), col(9), col(10), col(11)
                nc.vector.tensor_sub(tx1, tx, thw)
                nc.vector.tensor_add(tx2, tx, thw)
                nc.vector.tensor_sub(ty1, ty, thh)
                nc.vector.tensor_add(ty2, ty, thh)
                iw, ih = col(12), col(13)
                mn, mx = col(14), col(15)
                nc.vector.tensor_tensor(mn, px2, tx2, op=OP.min)
                nc.vector.tensor_tensor(mx, px1, tx1, op=OP.max)
                nc.vector.tensor_sub(iw, mn, mx)
                nc.vector.tensor_scalar_max(iw, iw, 0.0)
                nc.vector.tensor_tensor(mn, py2, ty2, op=OP.min)
                nc.vector.tensor_tensor(mx, py1, ty1, op=OP.max)
                nc.vector.tensor_sub(ih, mn, mx)
                nc.vector.tensor_scalar_max(ih, ih, 0.0)
                inter = col(16)
                nc.vector.tensor_mul(inter, iw, ih)
                uni, aa1 = col(17), col(18)
                nc.vector.tensor_mul(uni, pw, ph)
                nc.vector.tensor_mul(aa1, tw, th)
                nc.vector.tensor_add(uni, uni, aa1)
                nc.vector.tensor_sub(uni, uni, inter)
                nc.vector.tensor_scalar_add(uni, uni, EPS)
                iou = col(19)
                nc.vector.reciprocal(iou, uni)
                nc.vector.tensor_mul(iou, iou, inter)
                cw2, ch2 = col(20), col(21)
                nc.vector.tensor_tensor(mn, px2, tx2, op=OP.max)
                nc.vector.tensor_tensor(mx, px1, tx1, op=OP.min)
                nc.vector.tensor_sub(cw2, mn, mx)
                nc.vector.tensor_mul(cw2, cw2, cw2)
                nc.vector.tensor_tensor(mn, py2, ty2, op=OP.max)
                nc.vector.tensor_tensor(mx, py1, ty1, op=OP.min)
                nc.vector.tensor_sub(ch2, mn, mx)
                nc.vector.tensor_mul(ch2, ch2, ch2)
                nc.vector.tensor_add(cw2, cw2, ch2)
                nc.vector.tensor_scalar_add(cw2, cw2, EPS)  # c2
                rho2 = col(22)
                nc.vector.tensor_sub(mn, px, tx)
                nc.vector.tensor_mul(mn, mn, mn)
                nc.vector.tensor_sub(mx, py, ty)
                nc.vector.tensor_mul(mx, mx, mx)
                nc.vector.tensor_add(rho2, mn, mx)
                rc2 = col(23)
                nc.vector.reciprocal(rc2, cw2)
                nc.vector.tensor_mul(rho2, rho2, rc2)  # rho2/c2
                at = sp.tile([K, 24], dt.float32)
                q1, q2 = at[:, 0:3], at[:, 3:6]
                nc.vector.tensor_scalar_add(q1, th, EPS)
                nc.vector.reciprocal(q1, q1)
                nc.vector.tensor_mul(q1, q1, tw)
                nc.vector.tensor_scalar_add(q2, ph, EPS)
                nc.vector.reciprocal(q2, q2)
                nc.vector.tensor_mul(q2, q2, pw)

                # ---- focal batched on [K, 114] ----
                fe = sp.tile([K, 3 * CN], dt.float32)
                nc.scalar.activation(fe[:], gca[:], AF.Exp)
                fL = sp.tile([K, 3 * CN], dt.float32)
                nc.scalar.activation(fL[:], fe[:], AF.Ln, bias=1.0)
                fr = sp.tile([K, 3 * CN], dt.float32)
                nc.scalar.activation(fr[:], fL[:], AF.Exp, scale=-1.0)
                u = sp.tile([K, 3 * CN], dt.float32)
                nc.vector.tensor_scalar(out=u[:], in0=tca[:], scalar1=-2.0,
                                        scalar2=1.0, op0=OP.mult, op1=OP.add)
                q = sp.tile([K, 3 * CN], dt.float32)
                nc.vector.tensor_mul(q[:], fr[:], u[:])
                nc.vector.tensor_sub(q[:], u[:], q[:])
                nc.vector.tensor_add(q[:], q[:], tca[:])
                lq = sp.tile([K, 3 * CN], dt.float32)
                nc.scalar.activation(lq[:], q[:], AF.Ln)
                fw = sp.tile([K, 3 * CN], dt.float32)
                nc.scalar.activation(fw[:], lq[:], AF.Exp, scale=GAMMA)
                ce = sp.tile([K, 3 * CN], dt.float32)
                nc.vector.tensor_mul(ce[:], gca[:], tca[:])
                nc.vector.tensor_sub(ce[:], fL[:], ce[:])
                nc.vector.tensor_mul(fw[:], fw[:], ce[:])
                fj = sp.tile([K, CN], dt.float32)
                for i in range(3):
                    nc.vector.scalar_tensor_tensor(
                        out=fj[:], in0=fw[:, i*CN:(i+1)*CN], scalar=1.0,
                        in1=ama[:, i*CN:(i+1)*CN],
                        op0=OP.mult, op1=OP.mult,
                        accum_out=a_fc[:, i:i+1])

                # ---------------- hazard ----------------
                hx = sp.tile([BL, 4], dt.float32, tag="hx")
                nc.sync.dma_start(hx[:], haz_x[:])
                hoh = sp.tile([BL, 4], dt.float32, tag="hoh")
                nc.sync.dma_start(hoh[:], haz_oh[:])
                hpr = sp.tile([BL, 4], dt.float32, tag="hpr")
                nc.sync.dma_start(hpr[:], haz_pr[:])
                hm = sp.tile([BL, 8], dt.float32, tag="hm")
                mxc, nmx, Z, lz, s1, ce_, ec, hj = (
                    hm[:, m:m+1] for m in range(8))
                nc.vector.tensor_reduce(mxc, hx[:],
                                        axis=mybir.AxisListType.X, op=OP.max)
                nc.vector.tensor_scalar_mul(nmx, mxc, -1.0)
                he = sp.tile([BL, 4], dt.float32, tag="he")
                nc.scalar.activation(he[:], hx[:], AF.Exp, bias=nmx,
                                     accum_out=Z)
                nc.scalar.activation(lz, Z, AF.Ln)
                nc.vector.tensor_add(lz, lz, mxc)
                hj2 = sp.tile([BL, 4], dt.float32, tag="hj2")
                nc.vector.scalar_tensor_tensor(
                    out=hj2[:], in0=hx[:], scalar=1.0, in1=hoh[:],
                    op0=OP.mult, op1=OP.mult, accum_out=s1)
                nc.vector.tensor_sub(a_hz[:, 0:1], lz, s1)
                hj3 = sp.tile([BL, 4], dt.float32, tag="hj3")
                nc.vector.scalar_tensor_tensor(
                    out=hj3[:], in0=he[:], scalar=1.0, in1=hpr[:],
                    op0=OP.mult, op1=OP.mult, accum_out=ec)
                nc.vector.reciprocal(hj, Z)
                nc.vector.tensor_mul(ec, ec, hj)
                nc.vector.tensor_mul(a_hz[:, 1:2], ec, a_hz[:, 0:1])

                # ---- arctan tail (trig act table) + ciou finish ----
                a1v, a2v = at[:, 6:9], at[:, 9:12]
                nc.scalar.activation(a1v, q1, AF.Arctan)
                nc.scalar.activation(a2v, q2, AF.Arctan)
                dv, v = at[:, 12:15], at[:, 15:18]
                nc.vector.tensor_sub(dv, a1v, a2v)
                nc.vector.tensor_mul(v, dv, dv)
                nc.vector.tensor_scalar_mul(v, v, 4.0 / np.pi ** 2)
                den, al = at[:, 18:21], at[:, 21:24]
                nc.vector.tensor_scalar(out=den[:], in0=iou, scalar1=-1.0,
                                        scalar2=1.0 + EPS, op0=OP.mult,
                                        op1=OP.add)
                nc.vector.tensor_add(den, den, v)
                nc.vector.reciprocal(den, den)
                nc.vector.tensor_mul(al, v, den)
                nc.vector.tensor_mul(al, al, v)
                cio = at[:, 0:3]
                nc.vector.tensor_scalar(out=cio[:], in0=iou, scalar1=-1.0,
                                        scalar2=1.0, op0=OP.mult, op1=OP.add)
                nc.vector.tensor_add(cio, cio, rho2)
                nc.vector.tensor_add(cio, cio, al)
                nc.vector.tensor_tensor(out=a_ms[:, 3:6], in0=cio,
                                        in1=bma[:], op=OP.mult)

            # ---------------- assemble acc and store ----------------
            nc.vector.tensor_copy(acc[:, 0:8], a_sp[:])
            nc.vector.tensor_copy(acc[:, 8:16], a_r[:])
            nc.vector.tensor_copy(acc[:, 16:24], a_xt[:])
            nc.vector.tensor_copy(acc[:, 24:32], a_rt[:])
            nc.vector.tensor_copy(acc[0:1, 32:34], a_ts[:])
            nc.vector.tensor_copy(acc[:, 34:37], a_ob[:])
            if SEC in ("all", "sparse"):
                nc.vector.tensor_copy(acc[:, 37:40], a_ms[:, 0:3])
                nc.vector.tensor_copy(acc[:, 40:43], a_ms[:, 3:6])
                nc.vector.tensor_copy(acc[:, 43:46], a_fc[:])
                nc.vector.tensor_copy(acc[0:BL, 46:48], a_hz[:])
            nc.sync.dma_start(out[:], acc[:])

    nc.compile()
    return nc


def _jnp_targets(gt_boxes, gt_cls):
    """Replicate reference build_targets with jnp (oracle-exact semantics)."""
    import jax.numpy as jnp
    res = []
    for (H, W), (lo, hi) in zip(DIMS, SCALE_RANGES):
        gb = jnp.asarray(gt_boxes)
        gc = jnp.asarray(gt_cls)
        cx, cy, w, h = gb[..., 0], gb[..., 1], gb[..., 2], gb[..., 3]
        max_side = jnp.maximum(w, h)
        valid = (max_side >= lo) & (max_side < hi)
        gx = jnp.clip((cx * W).astype(jnp.int32), 0, W - 1)
        gy = jnp.clip((cy * H).astype(jnp.int32), 0, H - 1)
        flat = (jnp.arange(B, dtype=jnp.int32)[:, None] * (H * W)
                + gy * W + gx)
        flat = jnp.where(valid, flat, B * H * W).reshape(-1)
        tgt_reg = jnp.zeros((B * H * W + 1, 4), gb.dtype).at[flat].set(
            gb.reshape(-1, 4))[:B * H * W].reshape(B, H, W, 4)
        tgt_obj = jnp.zeros((B * H * W + 1,), gb.dtype).at[flat].set(
            1.0)[:B * H * W].reshape(B, H, W)
        tgt_cls = jnp.zeros((B * H * W + 1, NUM_CLASSES), gb.dtype).at[
            flat, gc.reshape(-1)].set(1.0)[:B * H * W].reshape(
            B, H, W, NUM_CLASSES)
        res.append((np.asarray(tgt_reg), np.asarray(tgt_obj),
                    np.asarray(tgt_cls)))
    return res


def kernel(**inputs):
    from concourse import bass_utils

    if "nc" not in _CACHE:
        _CACHE["nc"] = _build_program()
    nc = _CACHE["nc"]

    f32 = np.float32
    regs = [np.ascontiguousarray(inputs[f"reg_p{i+3}"], dtype=f32)
            for i in range(3)]
    objs = [np.ascontiguousarray(inputs[f"obj_p{i+3}"], dtype=f32)
            for i in range(3)]
    clss = [np.ascontiguousarray(inputs[f"cls_p{i+3}"], dtype=f32)
            for i in range(3)]
    targets = _jnp_targets(np.asarray(inputs["gt_boxes"], dtype=f32),
                           np.asarray(inputs["gt_cls"], dtype=np.int32))
    npos = [max(float(t[1].sum()), 1.0) for t in targets]

    proto = np.ascontiguousarray(inputs["proto_masks"], dtype=f32)
    gtm = np.ascontiguousarray(inputs["gt_masks"], dtype=f32)
    haz = np.ascontiguousarray(inputs["hazard_logits"], dtype=f32)
    ghz = np.asarray(inputs["gt_hazard"], dtype=np.int32)

    in_maps = []
    for core in range(NCORES):
        b0 = core * BL
        m = {
            "seg_x": proto[b0:b0+BL].reshape(BL, 128, SEG_F),
            "seg_t": gtm[b0:b0+BL].reshape(BL, 128, SEG_F),
            "objd3": objs[0][b0:b0+BL].reshape(128, 100),
            "objd4": objs[1][b0:b0+BL].reshape(128, 25),
            "objd5": objs[2][b0:b0+BL].reshape(100, 8),
            "haz_x": haz[b0:b0+BL],
            "haz_oh": np.eye(4, dtype=f32)[ghz[b0:b0+BL]],
            "haz_pr": PENALTY[ghz[b0:b0+BL]],
        }
        tbx_all = np.zeros((K, 12), f32)
        tcx_all = np.zeros((K, 3 * CN), f32)
        atm_all = np.zeros((K, 3 * CN), f32)
        bm_all = np.zeros((K, 3), f32)
        for i in range(3):
            HW = HWS[i]
            m[f"clst{i}"] = np.ascontiguousarray(
                clss[i][b0:b0+BL].transpose(0, 2, 3, 1).reshape(-1, CN))
            m[f"regt{i}"] = np.ascontiguousarray(
                regs[i][b0:b0+BL].transpose(0, 2, 3, 1).reshape(-1, 4))
            m[f"objf{i}"] = objs[i][b0:b0+BL].reshape(-1, 1)
            tgt_reg, tgt_obj, tgt_cls = targets[i]
            rows, tb, tcl = [], [], []
            for bl in range(BL):
                b = b0 + bl
                ys, xs = np.nonzero(tgt_obj[b])
                for gy, gx in zip(ys, xs):
                    rows.append(bl * HW + gy * DIMS[i][1] + gx)
                    tb.append(tgt_reg[b, gy, gx])
                    tcl.append(tgt_cls[b, gy, gx])
            n = len(rows)
            assert n <= K
            idx = np.zeros((K, 1), np.int32)
            if n:
                idx[:n, 0] = rows
                tbx = np.stack(tb)
                for ch in range(4):
                    tbx_all[:n, ch * 3 + i] = tbx[:, ch]
                tcx_all[:n, i*CN:(i+1)*CN] = np.stack(tcl)
                bm_all[:n, i] = 1.0
            m[f"idx{i}"] = idx
            atm_all[:, i*CN:(i+1)*CN] = (
                bm_all[:, i:i+1] * (0.75 - 0.5 * tcx_all[:, i*CN:(i+1)*CN]))
        m["tboxall"] = tbx_all
        m["tclsall"] = tcx_all
        m["atmall"] = atm_all
        m["bmaskall"] = bm_all
        in_maps.append(m)

    res = bass_utils.run_bass_kernel_spmd(nc, in_maps,
                                          core_ids=list(range(NCORES)))
    p = np.stack([res.results[c]["partials"] for c in range(NCORES)])
    p = p.astype(np.float64)

    # ---- host combine ----
    tb_ = to_ = tc_ = 0.0
    for i in range(3):
        H, W = DIMS[i]
        obj_sp = p[:, :, 34+i].sum()
        obj_pos = p[:, :, 37+i].sum()
        to_ += (obj_sp - obj_pos) / (B * H * W)
        tb_ += p[:, :, 40+i].sum() / npos[i]
        tc_ += p[:, :, 43+i].sum() / (npos[i] * NUM_CLASSES)
    tb_, to_, tc_ = tb_ / 3.0, to_ / 3.0, tc_ / 3.0

    sum_sp = p[:, :, 0:8].sum()
    sum_xt = p[:, :, 16:24].sum()
    bce = (sum_sp - sum_xt) / (B * SEG_ELEMS)
    dice = 0.0
    for sg in range(B):
        c, lo = sg // BL, sg % BL
        sum_r = p[c, :, 8+NCH*lo:8+NCH*(lo+1)].sum()
        P_b = SEG_ELEMS - sum_r
        T_b = p[c, 0, 32+lo]
        sum_rt = p[c, :, 24+NCH*lo:24+NCH*(lo+1)].sum()
        I_b = T_b - sum_rt
        dice += 1.0 - (2.0 * I_b + 1.0) / (P_b + T_b + 1.0)
    ts = (bce + dice / B) / 2.0

    th = p[:, 0:BL, 46].sum() / B
    thier = p[:, 0:BL, 47].sum() / B

    total = (L_BOX * tb_ + L_OBJ * to_ + L_CLS * tc_ + L_SEG * ts
             + L_HAZ * th + L_HIER * thier)
    return np.float32(total)



# revision 21
# speedup vs baseline: 2.0923x; 2.0923x over previous
"""Trainium2 Bass kernel for DBHDSNet multi-task detection loss.

Strategy (pure data parallel, B=16 over 8 cores, 2 samples/core):
 - Host (gt-only prep + zero-FLOP relayout): target assignment replicated
   with jnp (bit-exact with the reference's build_targets), per-core padded
   positive-cell tables; positive prediction rows pre-gathered into dense
   staging tiles (relayout only); per-sample gt-mask sums.
 - Device: all prediction-dependent math, ACT-minimal formulation:
   * seg (memory-bound bulk, 6.6MB/core in fp16): per chunk,
     sgm = Sigmoid(-x), then Ln(sgm) [accum -> -sum softplus(x)].
     Only 2 ACT transcendental passes per element.
   * dot products sum(x*t), sum(sgm*t) on the PE: 128x128 column-aligned
     matmul tiles accumulated into PSUM; the PSUM diagonal holds the
     per-column dots, extracted once with a masked-accumulate STT.
   * positives + obj: one combined Sigmoid over [-reg | cls | obj], one
     merged Ln over [fr | obj_sgm | q]; focal pow and hazard exp share
     one Exp op; CIoU batched over scales on DVE.
   * two ACT table phases (sigmoid/arctan, then ln/exp) -> 2 table loads.
 - Each core returns small partial tiles; host does the final scalar
   reduction (the "all-reduce" of scalar losses) in f64.
"""
import sys
sys.path.insert(0, "/opt/trn_rl_repo")
import numpy as np

NUM_CLASSES = 38
GAMMA, ALPHA = 1.5, 0.25
EPS = 1e-7
L_BOX, L_OBJ, L_CLS, L_SEG, L_HAZ, L_HIER = 5.0, 1.0, 1.0, 2.0, 1.0, 0.5
SCALE_RANGES = [(0.0, 0.15), (0.1, 0.35), (0.25, 1.0)]
PENALTY = np.array([[0., 1., 2., 4.], [2., 0., 1., 2.],
                    [4., 2., 0., 1.], [8., 4., 2., 0.]], dtype=np.float32)
B = 16
NCORES = 8
BL = B // NCORES
HWS = [6400, 1600, 400]
DIMS = [(80, 80), (40, 40), (20, 20)]
K = 128
SEG_F = 6400
CHUNK = 3200
NW = CHUNK // 128               # 128-col matmul tiles per chunk
NCH_S = SEG_F // CHUNK          # chunks per sample
NCH = BL * NCH_S                # chunks per core
SEG_ELEMS = 32 * 160 * 160
CN = NUM_CLASSES
OBJ_COLS = [100, 25, 7]         # per-scale col spans in objcat block
OBJ_TOT = sum(OBJ_COLS)
GW = 126 + OBJ_TOT              # gathobj width = 258

_CACHE = {}


def _build_program():
    import concourse.bacc as bacc
    import concourse.mybir as mybir
    import concourse.tile as tile
    dt = mybir.dt
    AF = mybir.ActivationFunctionType
    OP = mybir.AluOpType

    # Route Exp/Ln to the one table that holds both, and Arctan to the
    # sigmoid table, so the act-table pass emits exactly one load per
    # phase (sigmoid/arctan phase, then ln/exp phase).
    from concourse.hw_specs import get_activation_tables as _gat

    def _patched_tables(arch):
        tabs = _gat(arch)
        for name, s in tabs.items():
            if name != "natural_log_exp_and_others":
                s.discard(AF.Exp)
                s.discard(AF.Ln)
            if name != "sigmoid_and_others":
                s.discard(AF.Arctan)
        return tabs
    bacc.get_activation_tables = _patched_tables

    nc = bacc.Bacc("TRN2", target_bir_lowering=False, debug=False,
                   num_devices=NCORES)

    def din(name, shape, dty=dt.float32):
        return nc.dram_tensor(name, shape, dty, kind="ExternalInput").ap()

    seg_x = din("seg_x", [NCH, 128, CHUNK], dt.float16)
    seg_t = din("seg_t", [NCH, 128, CHUNK], dt.float8e4)
    gathobj = din("gathobj", [128, GW], dt.float16)
    tba = din("tba", [K, 12])          # target boxes, col = ch*3 + scale
    tca = din("tca", [K, 3 * CN])      # target cls onehot, scale-contiguous
    ama = din("ama", [K, 3 * CN])      # bm * (0.75 - 0.5*t)
    bma = din("bma", [K, 3])
    goa = din("goa", [K, 3])           # gathered obj logits at positives
    hazpk = din("hazpk", [BL, 12])     # [x | onehot | penalty_row]
    ident = din("ident", [128, 128])   # identity mask for PSUM diag

    def dout(name, shape):
        return nc.dram_tensor(name, shape, dt.float32,
                              kind="ExternalOutput").ap()

    o_segact = dout("o_segact", [128, 2 * NCH + 2])  # ln | dots | sgm sums
    o_obj = dout("o_obj", [128, 3])              # per-scale sum ln(sgm_obj)
    o_pos = dout("o_pos", [K, 3])                # goa * bma
    o_ciou = dout("o_ciou", [K, 3])              # ciou * bma
    o_fc = dout("o_fc", [K, 3])                  # sum fw*ce_neg*ama per scale
    o_haz = dout("o_haz", [BL, 2])               # [ce | ec*ce]

    with tile.TileContext(nc) as tc:
        with tc.tile_pool(name="xin", bufs=1) as xp, \
             tc.tile_pool(name="tin", bufs=1) as tp, \
             tc.tile_pool(name="sg", bufs=1) as gp, \
             tc.tile_pool(name="ja", bufs=2) as jap, \
             tc.tile_pool(name="jd", bufs=2) as jdp, \
             tc.tile_pool(name="small", bufs=1) as sp, \
             tc.tile_pool(name="psum", bufs=1, space="PSUM") as pp:

            # ---------------- DMAs in ----------------
            # x first (ACT-critical), x0 split for an early ACT start;
            # gathobj early (comb); t chunks next (PE); cold smalls last.
            xt = []
            for c in range(NCH):
                xt.append(xp.tile([128, CHUNK], dt.float16, tag=f"x{c}",
                                  name=f"x{c}"))
            tt = [tp.tile([128, CHUNK], dt.float8e4, tag=f"t{c}",
                          name=f"t{c}") for c in range(NCH)]
            H2 = CHUNK // 2
            nc.sync.dma_start(xt[0][:, 0:H2], seg_x[0, :, 0:H2])
            nc.sync.dma_start(xt[0][:, H2:CHUNK], seg_x[0, :, H2:CHUNK])
            nc.sync.dma_start(xt[1][:], seg_x[1, :, :])
            go2 = sp.tile([128, GW], dt.float16)
            nc.sync.dma_start(go2[:], gathobj[:])
            nc.sync.dma_start(tt[0][:], seg_t[0, :, :])
            nc.sync.dma_start(xt[2][:], seg_x[2, :, :])
            nc.sync.dma_start(xt[3][:], seg_x[3, :, :])
            nc.sync.dma_start(tt[1][:], seg_t[1, :, :])
            tb = sp.tile([K, 12], dt.float32)
            nc.sync.dma_start(tb[:], tba[:])
            tc_ = sp.tile([K, 3 * CN], dt.float32)
            nc.sync.dma_start(tc_[:], tca[:])
            hz = sp.tile([BL, 12], dt.float32)
            nc.sync.dma_start(hz[:], hazpk[:])
            idn = sp.tile([128, 128], dt.float32)
            nc.sync.dma_start(idn[:], ident[:])
            for c in range(2, NCH):
                nc.sync.dma_start(tt[c][:], seg_t[c, :, :])
            am = sp.tile([K, 3 * CN], dt.float32)
            nc.sync.dma_start(am[:], ama[:])
            bm = sp.tile([K, 3], dt.float32)
            nc.sync.dma_start(bm[:], bma[:])
            go = sp.tile([K, 3], dt.float32)
            nc.sync.dma_start(go[:], goa[:])

            # accumulator tiles
            # aact cols: 0:NCH ln-accums | NCH:NCH+4 prod dots | +2 sgm sums
            aact = sp.tile([128, 2 * NCH + 2], dt.float32)
            aobj = sp.tile([128, 3], dt.float32)
            afc = sp.tile([K, 3], dt.float32)
            ahaz = sp.tile([BL, 2], dt.float32)

            # ---------------- ACT phase 1: sigmoid table ----------------
            comb = sp.tile([K, 126 + OBJ_TOT + 114], dt.float16)  # [K,372]
            at = sp.tile([K, 24], dt.float32)
            sgm = [gp.tile([128, CHUNK], dt.float16, tag=f"s{c}",
                           name=f"s{c}") for c in range(NCH)]
            H = CHUNK // 2
            nc.scalar.activation(sgm[0][:, 0:H], xt[0][:, 0:H],
                                 AF.Sigmoid, scale=-1.0)
            nc.scalar.activation(sgm[0][:, H:CHUNK], xt[0][:, H:CHUNK],
                                 AF.Sigmoid, scale=-1.0)
            nc.scalar.activation(sgm[1][:], xt[1][:], AF.Sigmoid, scale=-1.0)
            nc.scalar.activation(sgm[2][:], xt[2][:], AF.Sigmoid, scale=-1.0)
            # [sig(reg) | sig(-cls) | sig(-obj)] = [dec | fr | osg]
            nc.scalar.activation(comb[:, 0:GW], go2[:], AF.Sigmoid,
                                 scale=-1.0)
            nc.scalar.activation(sgm[3][:], xt[3][:], AF.Sigmoid, scale=-1.0)

            # DVE early: sig accums c0/c1, pair-products, arctan args
            pr1 = []
            pr2 = []

            ones = sp.tile([128, 1], dt.float16)
            nc.vector.memset(ones[:], 1.0)

            def pairing(c):
                p1 = jap.tile([128, CHUNK // 2], dt.float16, tag="p1",
                              name="p1")
                nc.vector.tensor_tensor(p1[:], sgm[c][:, 0:CHUNK//2],
                                        sgm[c][:, CHUNK//2:CHUNK],
                                        op=OP.mult)
                p2 = jdp.tile([128, CHUNK // 4], dt.float32, tag="p2",
                              name="p2")
                nc.vector.tensor_tensor(p2[:], p1[:, 0:CHUNK//4],
                                        p1[:, CHUNK//4:CHUNK//2],
                                        op=OP.mult)
                pr2.append(p2)

            pairing(0)
            pairing(1)

            d4 = comb[:, 0:12]
            fr = comb[:, 12:126]
            w = sp.tile([K, 72], dt.float32)

            def col(n):
                return w[:, 3*n:3*n+3]
            px_, py_, pw_, ph_ = (d4[:, 3*m:3*m+3] for m in range(4))
            tx_, ty_, tw_, th_ = (tb[:, 3*m:3*m+3] for m in range(4))
            q1, q2 = at[:, 0:3], at[:, 3:6]
            nc.vector.tensor_scalar_add(q1, th_, EPS)
            nc.vector.reciprocal(q1, q1)
            nc.vector.tensor_mul(q1, q1, tw_)
            nc.vector.tensor_scalar_add(q2, ph_, EPS)
            nc.vector.reciprocal(q2, q2)
            nc.vector.tensor_mul(q2, q2, pw_)

            # ACT: single arctan over q1|q2 (still sigmoid-table phase)
            nc.scalar.activation(at[:, 6:12], at[:, 0:6], AF.Arctan)

            # DVE: focal q = (1-t) - fr*(1-2t) into comb[:, 258:372]
            u1 = sp.tile([K, 3 * CN], dt.float32)
            nc.vector.tensor_scalar(out=u1[:], in0=tc_[:], scalar1=-2.0,
                                    scalar2=1.0, op0=OP.mult, op1=OP.add)
            v1 = sp.tile([K, 3 * CN], dt.float32)
            nc.vector.tensor_scalar(out=v1[:], in0=tc_[:], scalar1=-1.0,
                                    scalar2=1.0, op0=OP.mult, op1=OP.add)
            m1 = sp.tile([K, 3 * CN], dt.float32)
            nc.vector.tensor_mul(m1[:], fr, u1[:])
            nc.vector.tensor_sub(comb[:, 258:372], v1[:], m1[:])

            # DVE: hazard prescale into zout tail; positives obj mask
            zout = sp.tile([K, 364], dt.float16)
            nc.vector.memset(zout[:, 360:364], 0.0)
            nc.vector.tensor_scalar_mul(zout[0:BL, 360:364], hz[:, 0:4],
                                        2.0 / 3.0)
            po = sp.tile([K, 3], dt.float32)
            nc.vector.tensor_tensor(out=po[:], in0=go[:], in1=bm[:],
                                    op=OP.mult)

            pairing(2)
            pairing(3)

            # ---------------- PE: dot products into PSUM diag ----------
            px = [pp.tile([128, 128], dt.float32, name=f"px{s}", tag=f"px{s}")
                  for s in range(BL)]
            ps = [pp.tile([128, 128], dt.float32, name=f"ps{s}", tag=f"ps{s}")
                  for s in range(BL)]
            pg = [pp.tile([128, 1], dt.float32, name=f"pg{s}", tag=f"pg{s}")
                  for s in range(BL)]
            for c in range(NCH):
                s = c // NCH_S
                first = (c % NCH_S == 0)
                last = (c % NCH_S == NCH_S - 1)
                for w_ in range(NW):
                    cs = slice(w_ * 128, (w_ + 1) * 128)
                    nc.tensor.matmul(px[s][:], lhsT=xt[c][:, cs],
                                     rhs=tt[c][:, cs],
                                     start=(first and w_ == 0),
                                     stop=(last and w_ == NW - 1),
                                     skip_group_check=True)
                for w_ in range(NW):
                    cs = slice(w_ * 128, (w_ + 1) * 128)
                    nc.tensor.matmul(ps[s][:], lhsT=sgm[c][:, cs],
                                     rhs=tt[c][:, cs],
                                     start=(first and w_ == 0),
                                     stop=(last and w_ == NW - 1),
                                     skip_group_check=True)
                for w_ in range(NW):
                    cs = slice(w_ * 128, (w_ + 1) * 128)
                    nc.tensor.matmul(pg[s][:], lhsT=sgm[c][:, cs],
                                     rhs=ones[:],
                                     start=(first and w_ == 0),
                                     stop=(last and w_ == NW - 1),
                                     skip_group_check=True)

            # ---------------- ACT phase 2: ln/exp table ----------------
            # merged Ln over [fr | osg | q] -> [lnfr | lnobj | lq]
            nc.scalar.activation(zout[:, 0:360], comb[:, 12:372], AF.Ln)
            # merged Exp: fw = exp(1.5 lq), hazard exp(x) (prescaled 2/3)
            fe = sp.tile([K, 118], dt.float32)
            nc.scalar.activation(fe[:], zout[:, 246:364], AF.Exp, scale=1.5)

            def seg_ln(c):
                jl = jap.tile([128, CHUNK // 4], dt.float32, tag="jl",
                              name="jl")
                nc.scalar.activation(jl[:], pr2[c][:], AF.Ln,
                                     accum_out=aact[:, c:c+1])

            seg_ln(0)
            # DVE: hazard Z reduce, then ACT lz
            zhz = sp.tile([BL, 4], dt.float32)
            nc.vector.tensor_reduce(zhz[:, 0:1], fe[0:BL, 114:118],
                                    axis=mybir.AxisListType.X, op=OP.add)
            seg_ln(1)
            nc.scalar.activation(zhz[:, 1:2], zhz[:, 0:1], AF.Ln)
            seg_ln(2)
            seg_ln(3)

            # ---------------- DVE: CIoU part 2 ----------------
            gv = nc.vector
            phw, phh, thw, thh = col(0), col(1), col(2), col(3)
            gv.tensor_scalar_mul(phw, pw_, 0.5)
            gv.tensor_scalar_mul(phh, ph_, 0.5)
            gv.tensor_scalar_mul(thw, tw_, 0.5)
            gv.tensor_scalar_mul(thh, th_, 0.5)
            px1, px2, py1, py2 = col(4), col(5), col(6), col(7)
            gv.tensor_sub(px1, px_, phw)
            gv.tensor_add(px2, px_, phw)
            gv.tensor_sub(py1, py_, phh)
            gv.tensor_add(py2, py_, phh)
            tx1, tx2, ty1, ty2 = col(8), col(9), col(10), col(11)
            gv.tensor_sub(tx1, tx_, thw)
            gv.tensor_add(tx2, tx_, thw)
            gv.tensor_sub(ty1, ty_, thh)
            gv.tensor_add(ty2, ty_, thh)
            iw, ih = col(12), col(13)
            mn, mx = col(14), col(15)
            gv.tensor_tensor(mn, px2, tx2, op=OP.min)
            gv.tensor_tensor(mx, px1, tx1, op=OP.max)
            gv.tensor_sub(iw, mn, mx)
            gv.tensor_scalar_max(iw, iw, 0.0)
            gv.tensor_tensor(mn, py2, ty2, op=OP.min)
            gv.tensor_tensor(mx, py1, ty1, op=OP.max)
            gv.tensor_sub(ih, mn, mx)
            gv.tensor_scalar_max(ih, ih, 0.0)
            inter = col(16)
            gv.tensor_mul(inter, iw, ih)
            uni, aa1 = col(17), col(18)
            gv.tensor_mul(uni, pw_, ph_)
            gv.tensor_mul(aa1, tw_, th_)
            gv.tensor_add(uni, uni, aa1)
            gv.tensor_sub(uni, uni, inter)
            gv.tensor_scalar_add(uni, uni, EPS)
            iou = col(19)
            nc.vector.reciprocal(iou, uni)
            gv.tensor_mul(iou, iou, inter)
            cw2, ch2 = col(20), col(21)
            gv.tensor_tensor(mn, px2, tx2, op=OP.max)
            gv.tensor_tensor(mx, px1, tx1, op=OP.min)
            gv.tensor_sub(cw2, mn, mx)
            gv.tensor_mul(cw2, cw2, cw2)
            gv.tensor_tensor(mn, py2, ty2, op=OP.max)
            gv.tensor_tensor(mx, py1, ty1, op=OP.min)
            gv.tensor_sub(ch2, mn, mx)
            gv.tensor_mul(ch2, ch2, ch2)
            gv.tensor_add(cw2, cw2, ch2)
            gv.tensor_scalar_add(cw2, cw2, EPS)  # c2
            rho2 = col(22)
            gv.tensor_sub(mn, px_, tx_)
            gv.tensor_mul(mn, mn, mn)
            gv.tensor_sub(mx, py_, ty_)
            gv.tensor_mul(mx, mx, mx)
            gv.tensor_add(rho2, mn, mx)
            rc2 = col(23)
            nc.vector.reciprocal(rc2, cw2)
            gv.tensor_mul(rho2, rho2, rc2)  # rho2/c2
            a1v, a2v = at[:, 6:9], at[:, 9:12]
            dv, v = at[:, 12:15], at[:, 15:18]
            gv.tensor_sub(dv, a1v, a2v)
            gv.tensor_mul(v, dv, dv)
            gv.tensor_scalar_mul(v, v, 4.0 / np.pi ** 2)
            den, al = at[:, 18:21], at[:, 21:24]
            gv.tensor_scalar(out=den[:], in0=iou, scalar1=-1.0,
                             scalar2=1.0 + EPS, op0=OP.mult, op1=OP.add)
            gv.tensor_add(den, den, v)
            nc.vector.reciprocal(den, den)
            gv.tensor_mul(al, v, den)
            gv.tensor_mul(al, al, v)
            cio = at[:, 0:3]
            gv.tensor_scalar(out=cio[:], in0=iou, scalar1=-1.0,
                             scalar2=1.0, op0=OP.mult, op1=OP.add)
            gv.tensor_add(cio, cio, rho2)
            gv.tensor_add(cio, cio, al)
            ciom = sp.tile([K, 3], dt.float32)
            gv.tensor_tensor(out=ciom[:], in0=cio, in1=bm[:], op=OP.mult)

            # ---------------- DVE tail ----------------
            # hazard dots
            hj = sp.tile([BL, 4], dt.float32)
            nc.vector.scalar_tensor_tensor(
                out=hj[:], in0=hz[:, 0:4], scalar=1.0, in1=hz[:, 4:8],
                op0=OP.mult, op1=OP.mult, accum_out=zhz[:, 2:3])
            hj2 = sp.tile([BL, 4], dt.float32)
            nc.vector.scalar_tensor_tensor(
                out=hj2[:], in0=fe[0:BL, 114:118], scalar=1.0,
                in1=hz[:, 8:12],
                op0=OP.mult, op1=OP.mult, accum_out=zhz[:, 3:4])

            # early diag readout (sample 0 PSUMs stop after chunk 1)
            def diag(pt, i):
                jdg = sp.tile([128, 128], dt.float32, tag=f"jdg{i}",
                              name=f"jdg{i}")
                nc.vector.scalar_tensor_tensor(
                    out=jdg[:], in0=pt[:], scalar=1.0, in1=idn[:],
                    op0=OP.mult, op1=OP.mult,
                    accum_out=aact[:, NCH+i:NCH+i+1])

            diag(px[0], 0)
            diag(ps[0], 2)
            nc.vector.tensor_copy(aact[:, 2*NCH:2*NCH+1], pg[0][:])

            # obj accums from zout slices
            c0 = 114
            for i, ncol in enumerate(OBJ_COLS):
                jo = sp.tile([K, ncol], dt.float32, tag=f"jo{i}",
                             name=f"jo{i}")
                nc.vector.tensor_scalar(out=jo[:],
                                        in0=zout[:, c0:c0+ncol],
                                        scalar1=1.0, scalar2=0.0,
                                        op0=OP.mult, op1=OP.add,
                                        accum_out=aobj[:, i:i+1])
                c0 += ncol

            # focal: ce_neg = ln(fr) + cls*t; contrib = fw*ce_neg*ama
            m2 = sp.tile([K, 3 * CN], dt.float32)
            nc.vector.tensor_mul(m2[:], go2[:, 12:126], tc_[:])
            nc.vector.tensor_add(m2[:], m2[:], zout[:, 0:114])
            nc.vector.tensor_mul(m2[:], m2[:], fe[:, 0:114])
            fj = sp.tile([K, CN], dt.float32)
            for i in range(3):
                nc.vector.scalar_tensor_tensor(
                    out=fj[:], in0=m2[:, i*CN:(i+1)*CN], scalar=1.0,
                    in1=am[:, i*CN:(i+1)*CN],
                    op0=OP.mult, op1=OP.mult,
                    accum_out=afc[:, i:i+1])

            # hazard tail
            nc.vector.tensor_sub(ahaz[:, 0:1], zhz[:, 1:2], zhz[:, 2:3])
            rz = sp.tile([BL, 1], dt.float32)
            nc.vector.reciprocal(rz[:], zhz[:, 0:1])
            nc.vector.tensor_mul(rz[:], rz[:], zhz[:, 3:4])
            nc.vector.tensor_mul(ahaz[:, 1:2], rz[:], ahaz[:, 0:1])

            # late diag readout (sample 1 PSUMs stop after chunk 3)
            diag(px[1], 1)
            diag(ps[1], 3)
            nc.vector.tensor_copy(aact[:, 2*NCH+1:2*NCH+2], pg[1][:])

            # ---------------- DMAs out ----------------
            nc.sync.dma_start(o_pos[:], po[:])
            nc.sync.dma_start(o_haz[:], ahaz[:])
            nc.sync.dma_start(o_fc[:], afc[:])
            nc.sync.dma_start(o_ciou[:], ciom[:])
            nc.sync.dma_start(o_obj[:], aobj[:])
            nc.sync.dma_start(o_segact[:], aact[:])

    nc.compile()
    return nc



def _jnp_targets(gt_boxes, gt_cls):
    """Replicate reference build_targets with jnp (oracle-exact semantics)."""
    import jax.numpy as jnp
    res = []
    for (H, W), (lo, hi) in zip(DIMS, SCALE_RANGES):
        gb = jnp.asarray(gt_boxes)
        gc = jnp.asarray(gt_cls)
        cx, cy, w, h = gb[..., 0], gb[..., 1], gb[..., 2], gb[..., 3]
        max_side = jnp.maximum(w, h)
        valid = (max_side >= lo) & (max_side < hi)
        gx = jnp.clip((cx * W).astype(jnp.int32), 0, W - 1)
        gy = jnp.clip((cy * H).astype(jnp.int32), 0, H - 1)
        flat = (jnp.arange(B, dtype=jnp.int32)[:, None] * (H * W)
                + gy * W + gx)
        flat = jnp.where(valid, flat, B * H * W).reshape(-1)
        tgt_reg = jnp.zeros((B * H * W + 1, 4), gb.dtype).at[flat].set(
            gb.reshape(-1, 4))[:B * H * W].reshape(B, H, W, 4)
        tgt_obj = jnp.zeros((B * H * W + 1,), gb.dtype).at[flat].set(
            1.0)[:B * H * W].reshape(B, H, W)
        tgt_cls = jnp.zeros((B * H * W + 1, NUM_CLASSES), gb.dtype).at[
            flat, gc.reshape(-1)].set(1.0)[:B * H * W].reshape(
            B, H, W, NUM_CLASSES)
        res.append((np.asarray(tgt_reg), np.asarray(tgt_obj),
                    np.asarray(tgt_cls)))
    return res


def kernel(**inputs):
    from concourse import bass_utils

    if "nc" not in _CACHE:
        _CACHE["nc"] = _build_program()
    nc = _CACHE["nc"]

    f32 = np.float32
    regs = [np.asarray(inputs[f"reg_p{i+3}"], dtype=f32) for i in range(3)]
    objs = [np.asarray(inputs[f"obj_p{i+3}"], dtype=f32) for i in range(3)]
    clss = [np.asarray(inputs[f"cls_p{i+3}"], dtype=f32) for i in range(3)]
    targets = _jnp_targets(np.asarray(inputs["gt_boxes"], dtype=f32),
                           np.asarray(inputs["gt_cls"], dtype=np.int32))
    npos = [max(float(t[1].sum()), 1.0) for t in targets]

    proto = np.asarray(inputs["proto_masks"], dtype=f32)
    gtm = np.asarray(inputs["gt_masks"], dtype=f32)
    haz = np.asarray(inputs["hazard_logits"], dtype=f32)
    ghz = np.asarray(inputs["gt_hazard"], dtype=np.int32)

    proto16 = proto.reshape(B, 128, NCH_S, CHUNK).transpose(
        0, 2, 1, 3).astype(np.float16)          # [B, NCH_S, 128, CHUNK]
    import concourse.mybir as _mb
    f8 = _mb.dt.np(_mb.dt.float8e4)
    gtm16 = gtm.reshape(B, 128, NCH_S, CHUNK).transpose(
        0, 2, 1, 3).astype(f8)
    t_sums = gtm.reshape(B, -1).astype(np.float64).sum(axis=1)
    identity = np.eye(128, dtype=f32)

    in_maps = []
    for core in range(NCORES):
        b0 = core * BL
        # obj cols carry +logits: device comb = sigmoid(-gathobj) gives
        # sigmoid(-obj) whose ln is -softplus(obj); pads -20 -> ln(1)=0
        gob = np.full((128, GW), -20.0, np.float16)
        gob[:, 126:226] = objs[0][b0:b0+BL].reshape(128, 100)
        gob[:, 226:251] = objs[1][b0:b0+BL].reshape(128, 25)
        o5 = objs[2][b0:b0+BL].reshape(-1)          # 800 values
        pad5 = np.full(128 * 7, -20.0, f32)
        pad5[:800] = o5
        gob[:, 251:258] = pad5.reshape(128, 7)
        m = {
            "seg_x": np.ascontiguousarray(
                proto16[b0:b0+BL].reshape(NCH, 128, CHUNK)),
            "seg_t": np.ascontiguousarray(
                gtm16[b0:b0+BL].reshape(NCH, 128, CHUNK)),
            "hazpk": np.concatenate(
                [haz[b0:b0+BL], np.eye(4, dtype=f32)[ghz[b0:b0+BL]],
                 PENALTY[ghz[b0:b0+BL]]], axis=1),
            "ident": identity,
        }
        tbx_all = np.zeros((K, 12), f32)
        tcx_all = np.zeros((K, 3 * CN), f32)
        atm_all = np.zeros((K, 3 * CN), f32)
        bm_all = np.zeros((K, 3), f32)
        goa_all = np.zeros((K, 3), f32)
        gathm = np.zeros((K, 126), f32)
        for i in range(3):
            tgt_reg, tgt_obj, tgt_cls = targets[i]
            regc = regs[i][b0:b0+BL]       # [BL,4,H,W]
            clsc = clss[i][b0:b0+BL]       # [BL,CN,H,W]
            objc_i = objs[i][b0:b0+BL]     # [BL,1,H,W]
            n = 0
            for bl in range(BL):
                bgl = b0 + bl
                ys, xs = np.nonzero(tgt_obj[bgl])
                for gy, gx in zip(ys, xs):
                    for ch in range(4):
                        gathm[n, ch * 3 + i] = -regc[bl, ch, gy, gx]
                        tbx_all[n, ch * 3 + i] = tgt_reg[bgl, gy, gx, ch]
                    gathm[n, 12 + i*CN:12 + (i+1)*CN] = clsc[bl, :, gy, gx]
                    tcx_all[n, i*CN:(i+1)*CN] = tgt_cls[bgl, gy, gx]
                    goa_all[n, i] = objc_i[bl, 0, gy, gx]
                    bm_all[n, i] = 1.0
                    n += 1
            assert n <= K
            atm_all[:, i*CN:(i+1)*CN] = (
                bm_all[:, i:i+1] * (0.75 - 0.5 * tcx_all[:, i*CN:(i+1)*CN]))
        # reg cols pre-negated so sigmoid(-(-reg)) = sigmoid(reg) decodes;
        # cls cols raw so sigmoid(-cls) = fr
        gob[:, 0:126] = gathm.astype(np.float16)
        m["gathobj"] = gob
        m["tba"] = tbx_all
        m["tca"] = tcx_all
        m["ama"] = atm_all
        m["bma"] = bm_all
        m["goa"] = goa_all
        in_maps.append(m)

    res = bass_utils.run_bass_kernel_spmd(nc, in_maps,
                                          core_ids=list(range(NCORES)))
    R = [res.results[c] for c in range(NCORES)]

    # ---- host combine (f64) ----
    f64 = np.float64
    sp_sum = 0.0       # total sum softplus(x) over seg
    xt_sum = 0.0
    dice = 0.0
    for c in range(NCORES):
        sa = R[c]["o_segact"].astype(f64)   # [128, 2*NCH+2]
        sp_sum += -sa[:, 0:NCH].sum()
        xt_sum += sa[:, NCH:NCH+BL].sum()
        for s in range(BL):
            bgl = c * BL + s
            one_m_sig = sa[:, 2*NCH + s].sum()
            P_b = SEG_ELEMS - one_m_sig
            T_b = t_sums[bgl]
            smt = sa[:, NCH + BL + s].sum()
            I_b = T_b - smt
            dice += 1.0 - (2.0 * I_b + 1.0) / (P_b + T_b + 1.0)
    bce = (sp_sum - xt_sum) / (B * SEG_ELEMS)
    ts = (bce + dice / B) / 2.0

    tb_ = to_ = tc2 = 0.0
    for i in range(3):
        H, W = DIMS[i]
        sp_obj = -sum(R[c]["o_obj"][:, i].astype(f64).sum()
                      for c in range(NCORES))
        pos_obj = sum(R[c]["o_pos"][:, i].astype(f64).sum()
                      for c in range(NCORES))
        to_ += (sp_obj - pos_obj) / (B * H * W)
        tb_ += sum(R[c]["o_ciou"][:, i].astype(f64).sum()
                   for c in range(NCORES)) / npos[i]
        tc2 += -sum(R[c]["o_fc"][:, i].astype(f64).sum()
                    for c in range(NCORES)) / (npos[i] * NUM_CLASSES)
    tb_, to_, tc2 = tb_ / 3.0, to_ / 3.0, tc2 / 3.0

    th = sum(R[c]["o_haz"][:, 0].astype(f64).sum()
             for c in range(NCORES)) / B
    thier = sum(R[c]["o_haz"][:, 1].astype(f64).sum()
                for c in range(NCORES)) / B

    total = (L_BOX * tb_ + L_OBJ * to_ + L_CLS * tc2 + L_SEG * ts
             + L_HAZ * th + L_HIER * thier)
    return np.float32(total)
